# revision 1
# baseline (speedup 1.0000x reference)
"""LocallyConnected2D (B=16, 32x32, CIN=COUT=64, 3x3, pad=1) on 8 TRN2 NeuronCores.

Strategy: shard the 32 output rows across 8 cores (4 rows each); weights are
repacked on the host into a per-core, DMA-friendly layout (128-partition,
fully contiguous). Per output pixel: 6 PSUM-accumulating matmuls
(3x K=128 covering kernel-rows {0,1} paired by equal column shift, 3x K=64
for kernel-row 2), M=16 (batch), N=64 (cout); 4 pixels run concurrently in
the PE array via column tile_position. Bias is added on the host.

out[b,i,j,o] = sum_{c,k} x_pad[b, i+di, j+dj, c] * W[o,c,i,j,k], k=3*di+dj.

Host layouts (per core c, local row r, i = 4c+r):
  w_pairs [4, 128, 6144]: [r, 64m+cin, o*96 + j*3 + t] = W[o, cin, i, j, 3m+t]
  w_sing  [4,  64, 6144]: [r, cin,     o*96 + j*3 + s] = W[o, cin, i, j, 6+s]
  xt      [6,  64,  512]: [rin, cin, j*16+b] = x_pad[b, 4c+rin, j, cin]
  out     [4, 4, 16, 8, 64]: [r, jj, b, g, o] = out[b, i, 4g+jj, o]
"""

import numpy as np

B, IH, IW, CIN = 16, 32, 32, 64
COUT, OH, OW = 64, 32, 32
NCORES, RPC = 8, 4

_NC = None


def _build_nc():
    import concourse.bacc as bacc
    import concourse.mybir as mybir
    import concourse.tile as tile

    f32 = mybir.dt.float32
    nc = bacc.Bacc("TRN2", target_bir_lowering=False, debug=False)
    wp = nc.dram_tensor("w_pairs", [RPC, 128, 6144], f32, kind="ExternalInput")
    ws = nc.dram_tensor("w_sing", [RPC, 64, 6144], f32, kind="ExternalInput")
    xt = nc.dram_tensor("xt", [RPC + 2, 64, 512], f32, kind="ExternalInput")
    out = nc.dram_tensor("out", [RPC, 4, 16, 8, 64], f32, kind="ExternalOutput")
    wp_ap, ws_ap, xt_ap, out_ap = wp.ap(), ws.ap(), xt.ap(), out.ap()

    with tile.TileContext(nc) as tc:
        with (
            tc.tile_pool(name="wp", bufs=2) as wp_pool,
            tc.tile_pool(name="ws", bufs=2) as ws_pool,
            tc.tile_pool(name="xd", bufs=2) as xd_pool,
            tc.tile_pool(name="xs", bufs=2) as xs_pool,
            tc.tile_pool(name="stage", bufs=2) as stage_pool,
            tc.tile_pool(name="psum", bufs=4, space="PSUM") as psum_pool,
        ):
            for r in range(RPC):
                wp_t = wp_pool.tile([128, 6144], f32, tag="wp")
                nc.sync.dma_start(wp_t[:], wp_ap[r])
                ws_t = ws_pool.tile([64, 6144], f32, tag="ws")
                nc.sync.dma_start(ws_t[:], ws_ap[r])

                # dual-row x tile: input rows (r, r+1) on partition halves;
                # free = (j'+1)*16 + b with one zero column pad on each side
                xd = xd_pool.tile([128, 544], f32, tag="xd")
                nc.gpsimd.memset(xd[:, 0:16], 0.0)
                nc.gpsimd.memset(xd[:, 528:544], 0.0)
                nc.sync.dma_start(xd[0:64, 16:528], xt_ap[r])
                nc.sync.dma_start(xd[64:128, 16:528], xt_ap[r + 1])
                xs = xs_pool.tile([64, 544], f32, tag="xs")
                nc.gpsimd.memset(xs[:, 0:16], 0.0)
                nc.gpsimd.memset(xs[:, 528:544], 0.0)
                nc.sync.dma_start(xs[0:64, 16:528], xt_ap[r + 2])

                stage = stage_pool.tile([128, 512], f32, tag="stage")
                wp_v = wp_t[:].rearrange("p (o q) -> p o q", q=96)
                ws_v = ws_t[:].rearrange("p (o q) -> p o q", q=96)

                for g in range(8):
                    ps = psum_pool.tile([128, 64], f32, tag="ps")
                    for t in range(6):
                        for jj in range(4):
                            j = 4 * g + jj
                            if t < 3:
                                lhsT = xd[:, (j + t) * 16 : (j + t + 1) * 16]
                                rhs = wp_v[:, :, 3 * j + t]
                            else:
                                s = t - 3
                                lhsT = xs[0:64, (j + s) * 16 : (j + s + 1) * 16]
                                rhs = ws_v[:, :, 3 * j + s]
                            nc.tensor.matmul(
                                ps[32 * jj : 32 * jj + 16, :],
                                lhsT,
                                rhs,
                                start=(t == 0),
                                stop=(t == 5),
                                tile_position=(0, 32 * jj),
                                skip_group_check=True,
                            )
                    for jj in range(4):
                        nc.vector.tensor_copy(
                            stage[32 * jj : 32 * jj + 16, g * 64 : (g + 1) * 64],
                            ps[32 * jj : 32 * jj + 16, :],
                        )
                for jj in range(4):
                    src = stage[32 * jj : 32 * jj + 16, :].rearrange(
                        "p (g o) -> p g o", o=64
                    )
                    nc.sync.dma_start(out_ap[r][jj], src)
    nc.compile()
    return nc


def _repack_inputs(x, weight):
    x = np.ascontiguousarray(np.asarray(x, dtype=np.float32))
    weight = np.ascontiguousarray(np.asarray(weight, dtype=np.float32))

    wt = np.ascontiguousarray(weight.transpose(2, 1, 0, 3, 4))  # [i, c, o, j, k]
    a = wt[..., :6].reshape(OH, CIN, COUT, OW, 2, 3)  # [i,c,o,j,m,t]
    wp = np.ascontiguousarray(a.transpose(0, 4, 1, 2, 3, 5)).reshape(OH, 128, 6144)
    ws = np.ascontiguousarray(wt[..., 6:9]).reshape(OH, CIN, 6144)

    xpad = np.zeros((IH + 2, CIN, IW, B), dtype=np.float32)
    xpad[1:33] = x.transpose(1, 3, 2, 0)  # [ih, c, j, b]

    in_maps = []
    for c in range(NCORES):
        in_maps.append(
            {
                "w_pairs": np.ascontiguousarray(wp[c * RPC : (c + 1) * RPC]),
                "w_sing": np.ascontiguousarray(ws[c * RPC : (c + 1) * RPC]),
                "xt": np.ascontiguousarray(
                    xpad[c * RPC : c * RPC + RPC + 2].reshape(RPC + 2, CIN, 512)
                ),
            }
        )
    return in_maps


def _get_nc():
    global _NC
    if _NC is None:
        _NC = _build_nc()
    return _NC


def run_spmd(in_maps, **kwargs):
    from concourse.bass_utils import run_bass_kernel_spmd

    return run_bass_kernel_spmd(
        _get_nc(), in_maps, core_ids=list(range(NCORES)), **kwargs
    )


def kernel(x, weight, bias, _results=None):
    if _results is None:
        _results = run_spmd(_repack_inputs(x, weight)).results
    arr = np.stack([r["out"] for r in _results])  # [core, r, jj, b, g, o]
    out = arr.transpose(3, 0, 1, 4, 2, 5).reshape(B, OH, OW, COUT)
    return out + np.asarray(bias, dtype=np.float32)[None]
